# revision 24
# baseline (speedup 1.0000x reference)
"""Two-layer GCN encoder on 8 Trainium2 NeuronCores (Bass/Tile).

Math (per layer, PyG GCNConv):
    deg[d]  = |{edges s->d}| + 1 (self loop)        [graph structure]
    dinv    = deg ** -0.5
    hs      = (dinv * x) @ W                        [= dinv * (x @ W)]
    agg[d]  = sum_{s in N(d)} hs[s] + hs[d]
    h       = relu(dinv * agg + b)                  [b == 0 here]
    out     = concat([h1, h2], axis=1)

Sharding: dst nodes are split evenly across the 8 cores.  Each core
computes hs for its own node shard (dense matmul), the shards are
AllGather'ed (in two half-shard chunks) into a replicated hs_full table
in DRAM, and each core pulls hs_full[src] for the non-self-loop edges
pointing into its shard with batched gather DMA (dma_gather, int16
indices over 25088-row source windows).  The four windows' gather calls
go to the four SWDGE queues so all 8 Q7 cores generate DMA descriptors
concurrently (descriptor generation, not DMA bandwidth, is the
bottleneck of this kernel).

Messages for one (span of 7 dst blocks, window) pair are packed
contiguously into 128-edge tiles sorted by dst block; per-core padding
is trailing (idx -1) so it generates no descriptors.  A 0/1 selection
matrix per (dst block, tile range), built on the vector engine by
comparing span-local dst slots against a per-block iota (fp16 so
integers up to 896 are exact), routes each tile through one PE matmul
that segment-sums messages into a PSUM accumulator per dst block.  The
self-loop contribution comes from a resident SBUF copy of the core's
own hs via an identity matmul (start=True).  Postprocessing is fused
into scalar-engine activations: h = relu(dinv * agg), and the layer-2
input s2 = dinv * h = relu(dinv^2 * agg).

Layer transition is pipelined: per span, s2 is stored, transposed back
into the (shared) xT tile with dma_start_transpose, and the span's
layer-2 hs matmuls run immediately; the two layer-2 AllGather chunks
fire as soon as each half shard's hs2 stores land.

Host-side work is limited to graph preprocessing: degree counts, edge
sorting, index layout, dtype casts.  All O(E*F) and O(N*F*F) floating
point work runs on the NeuronCores.
"""

import os

import ml_dtypes
import numpy as np

from concourse import bacc, bass, mybir
import concourse.tile as tile
from concourse.bass_utils import run_bass_kernel_spmd
from concourse.tile_rust import add_dep_helper
from concourse.library_config import mlp

BF16 = ml_dtypes.bfloat16
FP16 = np.float16
F32 = mybir.dt.float32
BF = mybir.dt.bfloat16
F16 = mybir.dt.float16
I16 = mybir.dt.int16

P = 128        # partitions / feature dim / edges per tile
SPAN = 7       # dst blocks per gather span
N_NODES = 100000
N_EDGES = 1600000
N_CORES = 8
FEAT = 128

NPC = N_NODES // N_CORES          # nodes per core (12500)
NBLK = -(-NPC // P)               # 128-node blocks per core (98)
NPCP = NBLK * P                   # padded nodes per core (12544)
NN = N_CORES * NPCP               # rows of the allgathered hs table (100352)
HALF = (NBLK // 2) * P            # rows per AllGather chunk (6272)
WROWS = NN // 4                   # int16 gather window (25088 < 32768)
NWIN = 4
NSPAN = -(-NBLK // SPAN)          # spans per core (14)


class Cfg:  # retained so test.py's K.run(K.CFG, ...) keeps working
    pass


CFG = Cfg()


def _ceil(a, b):
    return -(-a // b)


# ---------------------------------------------------------------------------
# Host-side graph preprocessing (indices only, plus dtype casts)
# ---------------------------------------------------------------------------

def prep_inputs(x, edge_index, W1, b1, W2, b2):
    x = np.asarray(x, dtype=np.float32)
    src = np.asarray(edge_index[0], dtype=np.int64)
    dst = np.asarray(edge_index[1], dtype=np.int64)

    deg = (np.bincount(dst, minlength=N_NODES) + 1).astype(np.float64)
    dinv = (1.0 / np.sqrt(deg)).astype(np.float32)

    # table row of node v: shards padded to NPCP, then split into half-shard
    # AllGather chunks: chunk k holds [core0 half_k, core1 half_k, ...]
    core_of = src // NPC
    loc = src % NPC
    half = (loc >= HALF).astype(np.int64)
    table_row = half * (NN // 2) + core_of * HALF + (loc - half * HALF)

    core_of_dst = dst // NPC

    ncall = NSPAN * NWIN
    per_core = []
    cnts = np.zeros((N_CORES, ncall), dtype=np.int64)
    # per-core cumulative message count by (span, window, block-within-span)
    cumh = np.zeros((N_CORES, ncall, SPAN + 1), dtype=np.int64)
    for c in range(N_CORES):
        m = core_of_dst == c
        srows = table_row[m]
        dloc = dst[m] - c * NPC
        bg = dloc >> 7
        s = bg // SPAN
        w = srows // WROWS
        key = s * NWIN + w
        order = np.lexsort((bg, key))
        srows, key, dloc, bg = srows[order], key[order], dloc[order], bg[order]
        cnts[c] = np.bincount(key, minlength=ncall)
        bin_sb = np.bincount(key * SPAN + (bg % SPAN),
                             minlength=ncall * SPAN).reshape(ncall, SPAN)
        cumh[c, :, 1:] = np.cumsum(bin_sb, axis=1)
        per_core.append((srows, key, dloc))

    # tiles per call: max over cores -> identical program on every core
    T_call = _ceil(cnts.max(axis=0), P)  # [ncall]
    gt0 = np.zeros(ncall + 1, dtype=np.int64)
    gt0[1:] = np.cumsum(T_call)
    TT = int(gt0[-1])

    # conservative per-(block, window) tile ranges shared by all cores
    # ranges[b][w] = (t0, t1) inclusive, or None
    ranges = [[None] * NWIN for _ in range(NBLK)]
    for s in range(NSPAN):
        b0 = s * SPAN
        for w in range(NWIN):
            call = s * NWIN + w
            for k in range(min(SPAN, NBLK - b0)):
                lo = int(cumh[:, call, k].min())
                hi = int(cumh[:, call, k + 1].max())
                if hi > lo:
                    ranges[b0 + k][w] = (int(gt0[call]) + lo // P,
                                         int(gt0[call]) + _ceil(hi, P) - 1)

    in_maps = []
    for c in range(N_CORES):
        srows, key, dloc = per_core[c]
        start = np.concatenate([[0], np.cumsum(cnts[c])[:-1]])
        pos = np.arange(len(key)) - start[key]
        gtile = gt0[key] + (pos >> 7)
        gpart = pos & 127

        # pad slots gather row 0 (harmless) and carry dst slot -1 (masked by
        # the is_equal selection matrix); avoids the negative-index strip path
        V = np.zeros((TT, P), np.int64)          # window-local source row
        D = np.full((TT, P), -1.0, np.float32)   # span-local dst slot
        V[gtile, gpart] = srows % WROWS
        D[gtile, gpart] = dloc - (key // NWIN) * (SPAN * P)

        # idx16 layout: per call the columns [8*gt0, 8*gt1); msg j (t-major)
        # lives at [16g + (j%16), gt0*8 + j//16], replicated to 128 partitions
        idx16 = np.zeros((P, TT * 8), np.int16)
        for call in range(ncall):
            a, b = int(gt0[call]), int(gt0[call + 1])
            if b == a:
                continue
            v = V[a:b, :].reshape(-1)
            blockv = v.reshape(-1, 16).T.astype(np.int16)
            idx16[:, a * 8:b * 8] = np.tile(blockv, (8, 1))

        xs = x[c * NPC:(c + 1) * NPC] * dinv[c * NPC:(c + 1) * NPC, None]
        xT = np.zeros((P, NPCP), np.float32)
        xT[:, :NPC] = xs.T
        dv = np.zeros(NPCP, np.float32)
        dv[:NPC] = dinv[c * NPC:(c + 1) * NPC]
        dinvT = np.ascontiguousarray(dv.reshape(NBLK, P).T)

        iot7 = np.broadcast_to(np.arange(SPAN * P, dtype=np.float32),
                               (P, SPAN * P)).copy()

        in_maps.append(
            {
                "xT": xT.astype(BF16),
                "idx16": idx16,
                "dsel": np.ascontiguousarray(D.T).astype(FP16),
                "dinvT": dinvT,
                "dinv2T": dinvT * dinvT,
                "w1": np.asarray(W1, np.float32).astype(BF16),
                "w2": np.asarray(W2, np.float32).astype(BF16),
                "iot7": iot7.astype(FP16),
                "ident": np.eye(P, dtype=np.float32).astype(BF16),
            }
        )
    return in_maps, T_call, ranges


# ---------------------------------------------------------------------------
# Device program
# ---------------------------------------------------------------------------

def build_program(T_call, ranges):
    n_f = FEAT
    gt0 = np.zeros(len(T_call) + 1, dtype=np.int64)
    gt0[1:] = np.cumsum(T_call)
    TT = int(gt0[-1])

    nc = bacc.Bacc("TRN2", target_bir_lowering=False, debug=False,
                   num_devices=N_CORES, num_swdge_queues=4,
                   dynamic_dma_scratch_size=24576)

    xT_d = nc.dram_tensor("xT", [P, NPCP], BF, kind="ExternalInput")
    idx16_d = nc.dram_tensor("idx16", [P, TT * 8], I16, kind="ExternalInput")
    dsel_d = nc.dram_tensor("dsel", [P, TT], F16, kind="ExternalInput")
    dinvT_d = nc.dram_tensor("dinvT", [P, NBLK], F32, kind="ExternalInput")
    dinv2T_d = nc.dram_tensor("dinv2T", [P, NBLK], F32, kind="ExternalInput")
    w_d = [nc.dram_tensor("w1", [n_f, n_f], BF, kind="ExternalInput"),
           nc.dram_tensor("w2", [n_f, n_f], BF, kind="ExternalInput")]
    iot7_d = nc.dram_tensor("iot7", [P, SPAN * P], F16, kind="ExternalInput")
    ident_d = nc.dram_tensor("ident", [P, P], BF, kind="ExternalInput")
    out_d = nc.dram_tensor("out", [NPC, 2 * n_f], F32, kind="ExternalOutput")

    s2_sh = nc.dram_tensor("s2sh", [NPCP, n_f], BF)
    hs_sh = [nc.dram_tensor(f"hs{L}sh", [NPCP, n_f], BF) for L in (1, 2)]
    hs_full = [nc.dram_tensor(f"hs{L}full", [NN, n_f], BF,
                              addr_space="Shared") for L in (1, 2)]
    groups = [list(range(N_CORES))]

    with tile.TileContext(nc) as tc:
        with (
            tc.tile_pool(name="const", bufs=1) as cpool,
            tc.tile_pool(name="big", bufs=1) as bigpool,
            tc.tile_pool(name="msg", bufs=3) as msgpool,
            tc.tile_pool(name="sel", bufs=8) as selpool,
            tc.tile_pool(name="post", bufs=4) as postpool,
            tc.tile_pool(name="psxw", bufs=2, space="PSUM") as psxw,
            tc.tile_pool(name="psag", bufs=6, space="PSUM") as psag,
        ):
            nc.gpsimd.load_library(mlp)
            w_t = []
            for L in (0, 1):
                wt = cpool.tile([n_f, n_f], BF, tag=f"w{L}", name=f"w{L}t")
                nc.sync.dma_start(out=wt[:], in_=w_d[L][:])
                w_t.append(wt)
            iot7_t = cpool.tile([P, SPAN * P], F16, tag="iot7", name="iot7_t")
            nc.sync.dma_start(out=iot7_t[:], in_=iot7_d[:])
            ident_t = cpool.tile([P, P], BF, tag="ident", name="ident_t")
            nc.sync.dma_start(out=ident_t[:], in_=ident_d[:])
            dinvT_t = cpool.tile([P, NBLK], F32, tag="dinvT", name="dinvT_t")
            nc.sync.dma_start(out=dinvT_t[:], in_=dinvT_d[:])
            dinv2T_t = cpool.tile([P, NBLK], F32, tag="dinv2T", name="dinv2T_t")
            nc.sync.dma_start(out=dinv2T_t[:], in_=dinv2T_d[:])

            # resident graph indices (shared by both layers)
            idx16_t = bigpool.tile([P, TT * 8], I16, tag="idx16", name="idx16_t")
            nc.sync.dma_start(out=idx16_t[:], in_=idx16_d[:])
            dsel_t = bigpool.tile([P, TT], F16, tag="dsel", name="dsel_t")
            nc.sync.dma_start(out=dsel_t[:], in_=dsel_d[:])

            # xT: layer-1 input, overwritten per span with transposed s2
            xT_t = bigpool.tile([P, NPCP], BF, tag="xT", name="xT_t")
            nc.sync.dma_start(out=xT_t[:], in_=xT_d[:])
            # resident own-shard hs (self-loop operand), overwritten per layer
            hso_t = bigpool.tile([P, NPCP], BF, tag="hso", name="hso_t")

            def xw_block(L, t):
                """hs_L[block t] = (xT[:, t].T @ W_L); store shard + SBUF copy.

                Layer-1 copies run on the (then idle) vector engine to shorten
                the startup ramp; layer-2 copies go to the scalar engine so
                they do not compete with the IS_EQ stream."""
                ps = psxw.tile([P, n_f], F32, tag="psxw", name="psxw_t")
                nc.tensor.matmul(out=ps[:], lhsT=xT_t[:, t * P:(t + 1) * P],
                                 rhs=w_t[L][:], start=True, stop=True)
                dst = hso_t[:, t * P:(t + 1) * P]
                if L == 0:
                    nc.vector.tensor_copy(out=dst, in_=ps[:])
                else:
                    nc.scalar.activation(out=dst, in_=ps[:],
                                         func=mybir.ActivationFunctionType.Copy)
                return nc.sync.dma_start(out=hs_sh[L][t * P:(t + 1) * P, :],
                                         in_=dst)

            def allgather_chunk(L, k, stores):
                ag = nc.gpsimd.collective_compute(
                    "AllGather", mybir.AluOpType.bypass, replica_groups=groups,
                    ins=[hs_sh[L][k * HALF:(k + 1) * HALF, :]],
                    outs=[hs_full[L][k * (NN // 2):(k + 1) * (NN // 2), :]])
                for s in stores:
                    add_dep_helper(ag.ins, s.ins, reason="allgather after hs stores")
                return ag

            glog = []  # gather instructions in emission order (for pinning)

            def span_gathers(L, s, ags, wins=range(NWIN), msg=None):
                """Issue window gather calls of span s (queue = window)."""
                t0 = int(gt0[s * NWIN])
                t1 = int(gt0[(s + 1) * NWIN])
                ts = t1 - t0
                if msg is None:
                    msg = msgpool.tile([P, ts, n_f], BF, tag="msg", name="msg_t")
                for w in wins:
                    a = int(gt0[s * NWIN + w])
                    b = int(gt0[s * NWIN + w + 1])
                    if b == a:
                        continue
                    nidx = (b - a) * P
                    g = nc.gpsimd.dma_gather(
                        msg[:, a - t0:b - t0, :],
                        hs_full[L][(w * WROWS):(w * WROWS + WROWS), :],
                        idx16_t[:, a * 8:b * 8],
                        nidx, nidx, n_f, single_packet=False, queue_num=w)
                    add_dep_helper(g.ins, ags[w // 2].ins,
                                   reason="gather after allgather chunk")
                    glog.append(g)
                return msg, t0

            def span_agg(L, s, msg, t0, s2_stores):
                """Segment-sum + postprocess the 7 blocks of span s."""
                b0 = s * SPAN
                for k in range(min(SPAN, NBLK - b0)):
                    b = b0 + k
                    rlist = [ranges[b][w] for w in range(NWIN)
                             if ranges[b][w] is not None]
                    ps = psag.tile([P, n_f], F32, tag="psag", name="psag_t")
                    nmm = sum(r1 - r0 + 1 for r0, r1 in rlist)
                    nc.tensor.matmul(out=ps[:], lhsT=ident_t[:],
                                     rhs=hso_t[:, b * P:(b + 1) * P],
                                     start=True, stop=(nmm == 0))
                    j = 0
                    for r0, r1 in rlist:
                        rn = r1 - r0 + 1
                        sel = selpool.tile([P, rn, P], BF, tag="sel",
                                           name="sel_t")
                        nc.vector.tensor_tensor(
                            out=sel[:],
                            in0=iot7_t[:, None, k * P:(k + 1) * P]
                                .to_broadcast([P, rn, P]),
                            in1=dsel_t[:, r0:r1 + 1, None]
                                .to_broadcast([P, rn, P]),
                            op=mybir.AluOpType.is_equal)
                        for t in range(rn):
                            nc.tensor.matmul(out=ps[:],
                                             lhsT=sel[:, t, :],
                                             rhs=msg[:, r0 + t - t0, :],
                                             start=False,
                                             stop=(j == nmm - 1))
                            j += 1
                    # h = relu(dinv * agg); s2 = dinv * h = relu(dinv^2 * agg)
                    h_t = postpool.tile([P, n_f], F32, tag="hrelu",
                                        name="hrelu_t")
                    nc.scalar.activation(out=h_t[:], in_=ps[:],
                                         func=mybir.ActivationFunctionType.Relu,
                                         scale=dinvT_t[:, b:b + 1])
                    rows = min(P, NPC - b * P)
                    nc.scalar.dma_start(
                        out=out_d[b * P:b * P + rows, L * n_f:(L + 1) * n_f],
                        in_=h_t[:rows, :])
                    if L == 0:
                        s2_t = postpool.tile([P, n_f], BF, tag="s2",
                                             name="s2_t")
                        nc.scalar.activation(
                            out=s2_t[:], in_=ps[:],
                            func=mybir.ActivationFunctionType.Relu,
                            scale=dinv2T_t[:, b:b + 1])
                        s2_stores.append(
                            nc.sync.dma_start(out=s2_sh[b * P:(b + 1) * P, :],
                                              in_=s2_t[:]))

            def span_xw2(s, s2_stores, hs2_stores):
                """Transpose span s's s2 back into xT and run its hs2 matmuls."""
                b0, b1 = s * SPAN, min((s + 1) * SPAN, NBLK)
                tr = nc.sync.dma_start_transpose(
                    out=xT_t[:, b0 * P:b1 * P],
                    in_=s2_sh[b0 * P:b1 * P, :])
                for st in s2_stores:
                    add_dep_helper(tr.ins, st.ins, reason="transpose after s2")
                for t in range(b0, b1):
                    hs2_stores.append(xw_block(1, t))

            # ---- layer 1 dense matmuls + chunked AllGather ----
            st1 = [xw_block(0, t) for t in range(NBLK)]
            ag1 = [allgather_chunk(0, 0, st1[:NBLK // 2]),
                   allgather_chunk(0, 1, st1[NBLK // 2:])]

            # ---- layer 1 aggregation, with layer-2 xw pipelined per span ----
            ag2 = [None, None]
            hs2_stores = []
            pending = []  # (s2_stores of span) awaiting span_xw2
            for s in range(NSPAN):
                msg, t0 = span_gathers(0, s, ag1)
                s2st = []
                span_agg(0, s, msg, t0, s2st)
                pending.append((s, s2st))
                # run xw2 for the previous span (keeps PE from stalling on
                # the s2 DRAM round-trip)
                if len(pending) > 1:
                    ps, pst = pending.pop(0)
                    span_xw2(ps, pst, hs2_stores)
            for ps, pst in pending:
                span_xw2(ps, pst, hs2_stores)

            # ---- layer 2 aggregation ----
            # Both AllGather chunks are emitted after the last layer-1
            # gathers (chunk 0's inputs are long since stored, so it only
            # costs its own execution, overlapped with layer-1 tail work).
            # The first two spans issue their window-0/1 gathers before any
            # window-2/3 gather so chunk 1 completes behind real gather work.
            # Both chunks pinned behind the last layer-1 gather (the scheduler
            # would otherwise hoist them mid-layer-1 and stall gather
            # dispatch); triggered back-to-back so their rendezvous/transfer
            # overlap as much as the CC hardware allows.
            last_l1 = glog[-1]
            ag2[0] = allgather_chunk(1, 0, hs2_stores[:NBLK // 2])
            add_dep_helper(ag2[0].ins, last_l1.ins,
                           reason="pin ag2[0] after last layer-1 gather")
            ag2[1] = allgather_chunk(1, 1, hs2_stores[NBLK // 2:])
            add_dep_helper(ag2[1].ins, last_l1.ins,
                           reason="pin ag2[1] after last layer-1 gather")
            m0, t00 = span_gathers(1, 0, ag2, wins=(0, 1))
            m1, t01 = span_gathers(1, 1, ag2, wins=(0, 1))
            span_gathers(1, 0, ag2, wins=(2, 3), msg=m0)
            span_gathers(1, 1, ag2, wins=(2, 3), msg=m1)
            span_agg(1, 0, m0, t00, [])
            span_agg(1, 1, m1, t01, [])
            for s in range(2, NSPAN):
                msg, t0 = span_gathers(1, s, ag2)
                span_agg(1, s, msg, t0, [])

    nc.compile()
    return nc


# ---------------------------------------------------------------------------
# Entry point
# ---------------------------------------------------------------------------

_CACHE: dict = {}


def _install_ntff_hook():
    """Wire the axon NTFF profiling hook that this image leaves unplugged.

    Harness-side instrumentation only; no-op when already present or
    when the pieces are missing."""
    try:
        from antenv.axon_hooks import get_axon_ntff_profile_hook  # noqa: F401
        return
    except ImportError:
        pass
    try:
        import sys
        import types

        if "/root/.axon_site" not in sys.path:
            sys.path.insert(0, "/root/.axon_site")
        from trn_agent_boot.trn_boot import _ntff_profile_via_ctypes

        hook = _ntff_profile_via_ctypes("/opt/axon/libaxon_pjrt.so")
        import antenv

        m = types.ModuleType("antenv.axon_hooks")
        m.get_axon_ntff_profile_hook = lambda: hook
        m.set_axon_ntff_profile_hook = lambda h: None
        sys.modules["antenv.axon_hooks"] = m
        antenv.axon_hooks = m
        import concourse.bass_utils as bu

        bu.upload_artifacts = lambda tmpdir: f"local:{tmpdir}"
    except Exception as e:  # degrade to no tracing
        print("ntff hook install failed:", e)


def run(cfg, inputs: dict, trace: bool = False):
    if trace:
        _install_ntff_hook()
    in_maps, T_call, ranges = prep_inputs(**inputs)
    key = (T_call.tobytes(), str(ranges))
    if key not in _CACHE:
        _CACHE[key] = build_program(T_call, ranges)
    nc = _CACHE[key]
    res = run_bass_kernel_spmd(nc, in_maps, list(range(N_CORES)), trace=trace)
    out = np.concatenate([res.results[c]["out"] for c in range(N_CORES)], axis=0)
    return out, res


def kernel(**inputs) -> np.ndarray:
    trace = bool(os.environ.get("BASS_TRACE"))
    try:
        out, _ = run(CFG, inputs, trace=trace)
    except Exception:
        # transient NRT / device hiccups happen rarely; one retry
        out, _ = run(CFG, inputs, trace=trace)
    return out


# revision 25
# speedup vs baseline: 1.1041x; 1.1041x over previous
"""Two-layer GCN encoder on 8 Trainium2 NeuronCores (Bass/Tile).

Math (per layer, PyG GCNConv):
    deg[d]  = |{edges s->d}| + 1 (self loop)        [graph structure]
    dinv    = deg ** -0.5
    hs      = (dinv * x) @ W                        [= dinv * (x @ W)]
    agg[d]  = sum_{s in N(d)} hs[s] + hs[d]
    h       = relu(dinv * agg + b)                  [b == 0 here]
    out     = concat([h1, h2], axis=1)

Sharding: dst nodes are split evenly across the 8 cores.  Each core
computes hs for its own node shard (dense matmul), the shards are
AllGather'ed (in two half-shard chunks) into a replicated hs_full table
in DRAM, and each core pulls hs_full[src] for the non-self-loop edges
pointing into its shard with batched gather DMA (dma_gather, int16
indices over 25088-row source windows).  The four windows' gather calls
go to the four SWDGE queues so all 8 Q7 cores generate DMA descriptors
concurrently (descriptor generation, not DMA bandwidth, is the
bottleneck of this kernel).

Messages for one (span of 7 dst blocks, window) pair are packed
contiguously into 128-edge tiles sorted by dst block; per-core padding
is trailing (idx -1) so it generates no descriptors.  A 0/1 selection
matrix per (dst block, tile range), built on the vector engine by
comparing span-local dst slots against a per-block iota (fp16 so
integers up to 896 are exact), routes each tile through one PE matmul
that segment-sums messages into a PSUM accumulator per dst block.  The
self-loop contribution comes from a resident SBUF copy of the core's
own hs via an identity matmul (start=True).  Postprocessing is fused
into scalar-engine activations: h = relu(dinv * agg), and the layer-2
input s2 = dinv * h = relu(dinv^2 * agg).

Layer transition is pipelined: per span, s2 is stored, transposed back
into the (shared) xT tile with dma_start_transpose, and the span's
layer-2 hs matmuls run immediately; the two layer-2 AllGather chunks
fire as soon as each half shard's hs2 stores land.

Host-side work is limited to graph preprocessing: degree counts, edge
sorting, index layout, dtype casts.  All O(E*F) and O(N*F*F) floating
point work runs on the NeuronCores.
"""

import os

import ml_dtypes
import numpy as np

from concourse import bacc, bass, mybir
import concourse.tile as tile
from concourse.bass_utils import run_bass_kernel_spmd
from concourse.tile_rust import add_dep_helper
from concourse.library_config import mlp

BF16 = ml_dtypes.bfloat16
FP16 = np.float16
F32 = mybir.dt.float32
BF = mybir.dt.bfloat16
F16 = mybir.dt.float16
I16 = mybir.dt.int16

P = 128        # partitions / feature dim / edges per tile
SPAN = 7       # dst blocks per gather span
N_NODES = 100000
N_EDGES = 1600000
N_CORES = 8
FEAT = 128

NPC = N_NODES // N_CORES          # nodes per core (12500)
NBLK = -(-NPC // P)               # 128-node blocks per core (98)
NPCP = NBLK * P                   # padded nodes per core (12544)
NN = N_CORES * NPCP               # rows of the allgathered hs table (100352)
HALF = (NBLK // 2) * P            # rows per AllGather chunk (6272)
WROWS = NN // 4                   # int16 gather window (25088 < 32768)
NWIN = 4
NSPAN = -(-NBLK // SPAN)          # spans per core (14)


class Cfg:  # retained so test.py's K.run(K.CFG, ...) keeps working
    pass


CFG = Cfg()


def _ceil(a, b):
    return -(-a // b)


# ---------------------------------------------------------------------------
# Host-side graph preprocessing (indices only, plus dtype casts)
# ---------------------------------------------------------------------------

def prep_inputs(x, edge_index, W1, b1, W2, b2):
    x = np.asarray(x, dtype=np.float32)
    src = np.asarray(edge_index[0], dtype=np.int64)
    dst = np.asarray(edge_index[1], dtype=np.int64)

    deg = (np.bincount(dst, minlength=N_NODES) + 1).astype(np.float64)
    dinv = (1.0 / np.sqrt(deg)).astype(np.float32)

    # table row of node v: shards padded to NPCP, then split into half-shard
    # AllGather chunks: chunk k holds [core0 half_k, core1 half_k, ...]
    core_of = src // NPC
    loc = src % NPC
    half = (loc >= HALF).astype(np.int64)
    table_row = half * (NN // 2) + core_of * HALF + (loc - half * HALF)

    core_of_dst = dst // NPC

    ncall = NSPAN * NWIN
    per_core = []
    cnts = np.zeros((N_CORES, ncall), dtype=np.int64)
    # per-core cumulative message count by (span, window, block-within-span)
    cumh = np.zeros((N_CORES, ncall, SPAN + 1), dtype=np.int64)
    for c in range(N_CORES):
        m = core_of_dst == c
        srows = table_row[m]
        dloc = dst[m] - c * NPC
        bg = dloc >> 7
        s = bg // SPAN
        w = srows // WROWS
        key = s * NWIN + w
        order = np.lexsort((bg, key))
        srows, key, dloc, bg = srows[order], key[order], dloc[order], bg[order]
        cnts[c] = np.bincount(key, minlength=ncall)
        bin_sb = np.bincount(key * SPAN + (bg % SPAN),
                             minlength=ncall * SPAN).reshape(ncall, SPAN)
        cumh[c, :, 1:] = np.cumsum(bin_sb, axis=1)
        per_core.append((srows, key, dloc))

    # tiles per call: max over cores -> identical program on every core
    T_call = _ceil(cnts.max(axis=0), P)  # [ncall]
    gt0 = np.zeros(ncall + 1, dtype=np.int64)
    gt0[1:] = np.cumsum(T_call)
    TT = int(gt0[-1])

    # conservative per-(block, window) tile ranges shared by all cores
    # ranges[b][w] = (t0, t1) inclusive, or None
    ranges = [[None] * NWIN for _ in range(NBLK)]
    for s in range(NSPAN):
        b0 = s * SPAN
        for w in range(NWIN):
            call = s * NWIN + w
            for k in range(min(SPAN, NBLK - b0)):
                lo = int(cumh[:, call, k].min())
                hi = int(cumh[:, call, k + 1].max())
                if hi > lo:
                    ranges[b0 + k][w] = (int(gt0[call]) + lo // P,
                                         int(gt0[call]) + _ceil(hi, P) - 1)

    in_maps = []
    for c in range(N_CORES):
        srows, key, dloc = per_core[c]
        start = np.concatenate([[0], np.cumsum(cnts[c])[:-1]])
        pos = np.arange(len(key)) - start[key]
        gtile = gt0[key] + (pos >> 7)
        gpart = pos & 127

        # pad slots gather row 0 (harmless) and carry dst slot -1 (masked by
        # the is_equal selection matrix); avoids the negative-index strip path
        V = np.zeros((TT, P), np.int64)          # window-local source row
        D = np.full((TT, P), -1.0, np.float32)   # span-local dst slot
        V[gtile, gpart] = srows % WROWS
        D[gtile, gpart] = dloc - (key // NWIN) * (SPAN * P)

        # idx16 layout: per call the columns [8*gt0, 8*gt1); msg j (t-major)
        # lives at [16g + (j%16), gt0*8 + j//16], replicated to 128 partitions
        idx16 = np.zeros((P, TT * 8), np.int16)
        for call in range(ncall):
            a, b = int(gt0[call]), int(gt0[call + 1])
            if b == a:
                continue
            v = V[a:b, :].reshape(-1)
            blockv = v.reshape(-1, 16).T.astype(np.int16)
            idx16[:, a * 8:b * 8] = np.tile(blockv, (8, 1))

        xs = x[c * NPC:(c + 1) * NPC] * dinv[c * NPC:(c + 1) * NPC, None]
        xT = np.zeros((P, NPCP), np.float32)
        xT[:, :NPC] = xs.T
        dv = np.zeros(NPCP, np.float32)
        dv[:NPC] = dinv[c * NPC:(c + 1) * NPC]
        dinvT = np.ascontiguousarray(dv.reshape(NBLK, P).T)

        iot7 = np.broadcast_to(np.arange(SPAN * P, dtype=np.float32),
                               (P, SPAN * P)).copy()

        in_maps.append(
            {
                "xT": xT.astype(BF16),
                "idx16": idx16,
                "dsel": np.ascontiguousarray(D.T).astype(FP16),
                "dinvT": dinvT,
                "dinv2T": dinvT * dinvT,
                "w1": np.asarray(W1, np.float32).astype(BF16),
                "w2": np.asarray(W2, np.float32).astype(BF16),
                "iot7": iot7.astype(FP16),
                "ident": np.eye(P, dtype=np.float32).astype(BF16),
            }
        )
    return in_maps, T_call, ranges


# ---------------------------------------------------------------------------
# Device program
# ---------------------------------------------------------------------------

def build_program(T_call, ranges):
    n_f = FEAT
    gt0 = np.zeros(len(T_call) + 1, dtype=np.int64)
    gt0[1:] = np.cumsum(T_call)
    TT = int(gt0[-1])

    nc = bacc.Bacc("TRN2", target_bir_lowering=False, debug=False,
                   num_devices=N_CORES, num_swdge_queues=4)

    xT_d = nc.dram_tensor("xT", [P, NPCP], BF, kind="ExternalInput")
    idx16_d = nc.dram_tensor("idx16", [P, TT * 8], I16, kind="ExternalInput")
    dsel_d = nc.dram_tensor("dsel", [P, TT], F16, kind="ExternalInput")
    dinvT_d = nc.dram_tensor("dinvT", [P, NBLK], F32, kind="ExternalInput")
    dinv2T_d = nc.dram_tensor("dinv2T", [P, NBLK], F32, kind="ExternalInput")
    w_d = [nc.dram_tensor("w1", [n_f, n_f], BF, kind="ExternalInput"),
           nc.dram_tensor("w2", [n_f, n_f], BF, kind="ExternalInput")]
    iot7_d = nc.dram_tensor("iot7", [P, SPAN * P], F16, kind="ExternalInput")
    ident_d = nc.dram_tensor("ident", [P, P], BF, kind="ExternalInput")
    out_d = nc.dram_tensor("out", [NPC, 2 * n_f], F32, kind="ExternalOutput")

    s2_sh = nc.dram_tensor("s2sh", [NPCP, n_f], BF)
    hs_sh = [nc.dram_tensor(f"hs{L}sh", [NPCP, n_f], BF) for L in (1, 2)]
    hs_full = [nc.dram_tensor(f"hs{L}full", [NN, n_f], BF,
                              addr_space="Shared") for L in (1, 2)]
    groups = [list(range(N_CORES))]

    with tile.TileContext(nc) as tc:
        with (
            tc.tile_pool(name="const", bufs=1) as cpool,
            tc.tile_pool(name="big", bufs=1) as bigpool,
            tc.tile_pool(name="msg", bufs=3) as msgpool,
            tc.tile_pool(name="sel", bufs=10) as selpool,
            tc.tile_pool(name="post", bufs=6) as postpool,
            tc.tile_pool(name="psxw", bufs=2, space="PSUM") as psxw,
            tc.tile_pool(name="psag", bufs=6, space="PSUM") as psag,
        ):
            nc.gpsimd.load_library(mlp)
            w_t = []
            for L in (0, 1):
                wt = cpool.tile([n_f, n_f], BF, tag=f"w{L}", name=f"w{L}t")
                nc.sync.dma_start(out=wt[:], in_=w_d[L][:])
                w_t.append(wt)
            iot7_t = cpool.tile([P, SPAN * P], F16, tag="iot7", name="iot7_t")
            nc.sync.dma_start(out=iot7_t[:], in_=iot7_d[:])
            ident_t = cpool.tile([P, P], BF, tag="ident", name="ident_t")
            nc.sync.dma_start(out=ident_t[:], in_=ident_d[:])
            dinvT_t = cpool.tile([P, NBLK], F32, tag="dinvT", name="dinvT_t")
            nc.sync.dma_start(out=dinvT_t[:], in_=dinvT_d[:])
            dinv2T_t = cpool.tile([P, NBLK], F32, tag="dinv2T", name="dinv2T_t")
            nc.sync.dma_start(out=dinv2T_t[:], in_=dinv2T_d[:])

            # resident graph indices (shared by both layers)
            idx16_t = bigpool.tile([P, TT * 8], I16, tag="idx16", name="idx16_t")
            nc.sync.dma_start(out=idx16_t[:], in_=idx16_d[:])
            dsel_t = bigpool.tile([P, TT], F16, tag="dsel", name="dsel_t")
            nc.sync.dma_start(out=dsel_t[:], in_=dsel_d[:])

            # xT: layer-1 input, overwritten per span with transposed s2
            xT_t = bigpool.tile([P, NPCP], BF, tag="xT", name="xT_t")
            nc.sync.dma_start(out=xT_t[:], in_=xT_d[:])
            # resident own-shard hs (self-loop operand), overwritten per layer
            hso_t = bigpool.tile([P, NPCP], BF, tag="hso", name="hso_t")

            def xw_block(L, t):
                """hs_L[block t] = (xT[:, t].T @ W_L); store shard + SBUF copy.

                Layer-1 copies run on the (then idle) vector engine to shorten
                the startup ramp; layer-2 copies go to the scalar engine so
                they do not compete with the IS_EQ stream."""
                ps = psxw.tile([P, n_f], F32, tag="psxw", name="psxw_t")
                nc.tensor.matmul(out=ps[:], lhsT=xT_t[:, t * P:(t + 1) * P],
                                 rhs=w_t[L][:], start=True, stop=True)
                dst = hso_t[:, t * P:(t + 1) * P]
                if L == 0:
                    nc.vector.tensor_copy(out=dst, in_=ps[:])
                else:
                    nc.scalar.activation(out=dst, in_=ps[:],
                                         func=mybir.ActivationFunctionType.Copy)
                return nc.sync.dma_start(out=hs_sh[L][t * P:(t + 1) * P, :],
                                         in_=dst)

            def allgather_chunk(L, k, stores):
                ag = nc.gpsimd.collective_compute(
                    "AllGather", mybir.AluOpType.bypass, replica_groups=groups,
                    ins=[hs_sh[L][k * HALF:(k + 1) * HALF, :]],
                    outs=[hs_full[L][k * (NN // 2):(k + 1) * (NN // 2), :]])
                for s in stores:
                    add_dep_helper(ag.ins, s.ins, reason="allgather after hs stores")
                return ag

            glog = []  # gather instructions in emission order (for pinning)

            def span_gathers(L, s, ags, wins=range(NWIN), msg=None):
                """Issue window gather calls of span s (queue = window)."""
                t0 = int(gt0[s * NWIN])
                t1 = int(gt0[(s + 1) * NWIN])
                ts = t1 - t0
                if msg is None:
                    msg = msgpool.tile([P, ts, n_f], BF, tag="msg", name="msg_t")
                for w in wins:
                    a = int(gt0[s * NWIN + w])
                    b = int(gt0[s * NWIN + w + 1])
                    if b == a:
                        continue
                    nidx = (b - a) * P
                    g = nc.gpsimd.dma_gather(
                        msg[:, a - t0:b - t0, :],
                        hs_full[L][(w * WROWS):(w * WROWS + WROWS), :],
                        idx16_t[:, a * 8:b * 8],
                        nidx, nidx, n_f, single_packet=False, queue_num=w)
                    add_dep_helper(g.ins, ags[w // 2].ins,
                                   reason="gather after allgather chunk")
                    glog.append(g)
                return msg, t0

            def span_agg(L, s, msg, t0, s2_stores):
                """Segment-sum + postprocess the 7 blocks of span s."""
                b0 = s * SPAN
                for k in range(min(SPAN, NBLK - b0)):
                    b = b0 + k
                    rlist = [ranges[b][w] for w in range(NWIN)
                             if ranges[b][w] is not None]
                    ps = psag.tile([P, n_f], F32, tag="psag", name="psag_t")
                    nmm = sum(r1 - r0 + 1 for r0, r1 in rlist)
                    nc.tensor.matmul(out=ps[:], lhsT=ident_t[:],
                                     rhs=hso_t[:, b * P:(b + 1) * P],
                                     start=True, stop=(nmm == 0))
                    j = 0
                    for r0, r1 in rlist:
                        rn = r1 - r0 + 1
                        sel = selpool.tile([P, rn, P], BF, tag="sel",
                                           name="sel_t")
                        nc.vector.tensor_tensor(
                            out=sel[:],
                            in0=iot7_t[:, None, k * P:(k + 1) * P]
                                .to_broadcast([P, rn, P]),
                            in1=dsel_t[:, r0:r1 + 1, None]
                                .to_broadcast([P, rn, P]),
                            op=mybir.AluOpType.is_equal)
                        for t in range(rn):
                            nc.tensor.matmul(out=ps[:],
                                             lhsT=sel[:, t, :],
                                             rhs=msg[:, r0 + t - t0, :],
                                             start=False,
                                             stop=(j == nmm - 1))
                            j += 1
                    # h = relu(dinv * agg); s2 = dinv * h = relu(dinv^2 * agg)
                    h_t = postpool.tile([P, n_f], F32, tag="hrelu",
                                        name="hrelu_t")
                    nc.scalar.activation(out=h_t[:], in_=ps[:],
                                         func=mybir.ActivationFunctionType.Relu,
                                         scale=dinvT_t[:, b:b + 1])
                    rows = min(P, NPC - b * P)
                    nc.scalar.dma_start(
                        out=out_d[b * P:b * P + rows, L * n_f:(L + 1) * n_f],
                        in_=h_t[:rows, :])
                    if L == 0:
                        s2_t = postpool.tile([P, n_f], BF, tag="s2",
                                             name="s2_t")
                        nc.scalar.activation(
                            out=s2_t[:], in_=ps[:],
                            func=mybir.ActivationFunctionType.Relu,
                            scale=dinv2T_t[:, b:b + 1])
                        s2_stores.append(
                            nc.sync.dma_start(out=s2_sh[b * P:(b + 1) * P, :],
                                              in_=s2_t[:]))

            def span_xw2(s, s2_stores, hs2_stores):
                """Transpose span s's s2 back into xT and run its hs2 matmuls."""
                b0, b1 = s * SPAN, min((s + 1) * SPAN, NBLK)
                tr = nc.sync.dma_start_transpose(
                    out=xT_t[:, b0 * P:b1 * P],
                    in_=s2_sh[b0 * P:b1 * P, :])
                for st in s2_stores:
                    add_dep_helper(tr.ins, st.ins, reason="transpose after s2")
                for t in range(b0, b1):
                    hs2_stores.append(xw_block(1, t))

            # ---- layer 1 dense matmuls + chunked AllGather ----
            st1 = [xw_block(0, t) for t in range(NBLK)]
            ag1 = [allgather_chunk(0, 0, st1[:NBLK // 2]),
                   allgather_chunk(0, 1, st1[NBLK // 2:])]

            # ---- layer 1 aggregation, with layer-2 xw pipelined per span ----
            ag2 = [None, None]
            hs2_stores = []
            pending = []  # (s2_stores of span) awaiting span_xw2
            for s in range(NSPAN):
                msg, t0 = span_gathers(0, s, ag1)
                s2st = []
                span_agg(0, s, msg, t0, s2st)
                pending.append((s, s2st))
                # run xw2 for the previous span (keeps PE from stalling on
                # the s2 DRAM round-trip)
                if len(pending) > 1:
                    ps, pst = pending.pop(0)
                    span_xw2(ps, pst, hs2_stores)
            for ps, pst in pending:
                span_xw2(ps, pst, hs2_stores)

            # ---- layer 2 aggregation ----
            # Both AllGather chunks are emitted after the last layer-1
            # gathers (chunk 0's inputs are long since stored, so it only
            # costs its own execution, overlapped with layer-1 tail work).
            # The first two spans issue their window-0/1 gathers before any
            # window-2/3 gather so chunk 1 completes behind real gather work.
            # Both chunks pinned behind the last layer-1 gather (the scheduler
            # would otherwise hoist them mid-layer-1 and stall gather
            # dispatch); triggered back-to-back so their rendezvous/transfer
            # overlap as much as the CC hardware allows.
            last_l1 = glog[-1]
            ag2[0] = allgather_chunk(1, 0, hs2_stores[:NBLK // 2])
            add_dep_helper(ag2[0].ins, last_l1.ins,
                           reason="pin ag2[0] after last layer-1 gather")
            ag2[1] = allgather_chunk(1, 1, hs2_stores[NBLK // 2:])
            add_dep_helper(ag2[1].ins, last_l1.ins,
                           reason="pin ag2[1] after last layer-1 gather")
            m0, t00 = span_gathers(1, 0, ag2, wins=(0, 1))
            m1, t01 = span_gathers(1, 1, ag2, wins=(0, 1))
            span_gathers(1, 0, ag2, wins=(2, 3), msg=m0)
            span_gathers(1, 1, ag2, wins=(2, 3), msg=m1)
            span_agg(1, 0, m0, t00, [])
            span_agg(1, 1, m1, t01, [])
            for s in range(2, NSPAN):
                msg, t0 = span_gathers(1, s, ag2)
                span_agg(1, s, msg, t0, [])

    nc.compile()
    return nc


# ---------------------------------------------------------------------------
# Entry point
# ---------------------------------------------------------------------------

_CACHE: dict = {}


def _install_ntff_hook():
    """Wire the axon NTFF profiling hook that this image leaves unplugged.

    Harness-side instrumentation only; no-op when already present or
    when the pieces are missing."""
    try:
        from antenv.axon_hooks import get_axon_ntff_profile_hook  # noqa: F401
        return
    except ImportError:
        pass
    try:
        import sys
        import types

        if "/root/.axon_site" not in sys.path:
            sys.path.insert(0, "/root/.axon_site")
        from trn_agent_boot.trn_boot import _ntff_profile_via_ctypes

        hook = _ntff_profile_via_ctypes("/opt/axon/libaxon_pjrt.so")
        import antenv

        m = types.ModuleType("antenv.axon_hooks")
        m.get_axon_ntff_profile_hook = lambda: hook
        m.set_axon_ntff_profile_hook = lambda h: None
        sys.modules["antenv.axon_hooks"] = m
        antenv.axon_hooks = m
        import concourse.bass_utils as bu

        bu.upload_artifacts = lambda tmpdir: f"local:{tmpdir}"
    except Exception as e:  # degrade to no tracing
        print("ntff hook install failed:", e)


def run(cfg, inputs: dict, trace: bool = False):
    if trace:
        _install_ntff_hook()
    in_maps, T_call, ranges = prep_inputs(**inputs)
    key = (T_call.tobytes(), str(ranges))
    if key not in _CACHE:
        _CACHE[key] = build_program(T_call, ranges)
    nc = _CACHE[key]
    res = run_bass_kernel_spmd(nc, in_maps, list(range(N_CORES)), trace=trace)
    out = np.concatenate([res.results[c]["out"] for c in range(N_CORES)], axis=0)
    return out, res


def kernel(**inputs) -> np.ndarray:
    trace = bool(os.environ.get("BASS_TRACE"))
    try:
        out, _ = run(CFG, inputs, trace=trace)
    except Exception:
        # transient NRT / device hiccups happen rarely; one retry
        out, _ = run(CFG, inputs, trace=trace)
    return out


# revision 26
# speedup vs baseline: 1.1264x; 1.0202x over previous
"""Two-layer GCN encoder on 8 Trainium2 NeuronCores (Bass/Tile).

Math (per layer, PyG GCNConv):
    deg[d]  = |{edges s->d}| + 1 (self loop)        [graph structure]
    dinv    = deg ** -0.5
    hs      = (dinv * x) @ W                        [= dinv * (x @ W)]
    agg[d]  = sum_{s in N(d)} hs[s] + hs[d]
    h       = relu(dinv * agg + b)                  [b == 0 here]
    out     = concat([h1, h2], axis=1)

Sharding: dst nodes are split evenly across the 8 cores.  Each core
computes hs for its own node shard (dense matmul), the shards are
AllGather'ed (in two half-shard chunks) into a replicated hs_full table
in DRAM, and each core pulls hs_full[src] for the non-self-loop edges
pointing into its shard with batched gather DMA (dma_gather, int16
indices over 25088-row source windows).  The four windows' gather calls
go to the four SWDGE queues so all 8 Q7 cores generate DMA descriptors
concurrently (descriptor generation, not DMA bandwidth, is the
bottleneck of this kernel).

Messages for one (span of 7 dst blocks, window) pair are packed
contiguously into 128-edge tiles sorted by dst block; per-core padding
is trailing (idx -1) so it generates no descriptors.  A 0/1 selection
matrix per (dst block, tile range), built on the vector engine by
comparing span-local dst slots against a per-block iota (fp16 so
integers up to 896 are exact), routes each tile through one PE matmul
that segment-sums messages into a PSUM accumulator per dst block.  The
self-loop contribution comes from a resident SBUF copy of the core's
own hs via an identity matmul (start=True).  Postprocessing is fused
into scalar-engine activations: h = relu(dinv * agg), and the layer-2
input s2 = dinv * h = relu(dinv^2 * agg).

Layer transition is pipelined: per span, s2 is stored, transposed back
into the (shared) xT tile with dma_start_transpose, and the span's
layer-2 hs matmuls run immediately; the two layer-2 AllGather chunks
are pinned behind the last layer-1 gather (the Tile scheduler would
otherwise hoist them mid-layer-1, stalling gather dispatch on the Pool
engine), and layer 2's first two spans issue their window-0/1 gathers
before any window-2/3 gather so the second chunk completes behind real
gather work.

Host-side work is limited to graph preprocessing: degree counts, edge
sorting, index layout, dtype casts.  All O(E*F) and O(N*F*F) floating
point work runs on the NeuronCores.
"""

import os

import ml_dtypes
import numpy as np

from concourse import bacc, bass, mybir
import concourse.tile as tile
from concourse.bass_utils import run_bass_kernel_spmd
from concourse.tile_rust import add_dep_helper
from concourse.library_config import mlp

BF16 = ml_dtypes.bfloat16
FP16 = np.float16
F32 = mybir.dt.float32
BF = mybir.dt.bfloat16
F16 = mybir.dt.float16
I16 = mybir.dt.int16

P = 128        # partitions / feature dim / edges per tile
SPAN = 7       # dst blocks per gather span
N_NODES = 100000
N_EDGES = 1600000
N_CORES = 8
FEAT = 128

NPC = N_NODES // N_CORES          # nodes per core (12500)
NBLK = -(-NPC // P)               # 128-node blocks per core (98)
NPCP = NBLK * P                   # padded nodes per core (12544)
NN = N_CORES * NPCP               # rows of the allgathered hs table (100352)
HALF = (NBLK // 2) * P            # rows per AllGather chunk (6272)
WROWS = NN // 4                   # int16 gather window (25088 < 32768)
NWIN = 4
NSPAN = -(-NBLK // SPAN)          # spans per core (14)


class Cfg:  # retained so test.py's K.run(K.CFG, ...) keeps working
    pass


CFG = Cfg()


def _ceil(a, b):
    return -(-a // b)


# ---------------------------------------------------------------------------
# Host-side graph preprocessing (indices only, plus dtype casts)
# ---------------------------------------------------------------------------

def prep_inputs(x, edge_index, W1, b1, W2, b2):
    x = np.asarray(x, dtype=np.float32)
    src = np.asarray(edge_index[0], dtype=np.int64)
    dst = np.asarray(edge_index[1], dtype=np.int64)

    deg = (np.bincount(dst, minlength=N_NODES) + 1).astype(np.float64)
    dinv = (1.0 / np.sqrt(deg)).astype(np.float32)

    # table row of node v: shards padded to NPCP, then split into half-shard
    # AllGather chunks: chunk k holds [core0 half_k, core1 half_k, ...]
    core_of = src // NPC
    loc = src % NPC
    half = (loc >= HALF).astype(np.int64)
    table_row = half * (NN // 2) + core_of * HALF + (loc - half * HALF)

    core_of_dst = dst // NPC

    ncall = NSPAN * NWIN
    per_core = []
    cnts = np.zeros((N_CORES, ncall), dtype=np.int64)
    # per-core cumulative message count by (span, window, block-within-span)
    cumh = np.zeros((N_CORES, ncall, SPAN + 1), dtype=np.int64)
    for c in range(N_CORES):
        m = core_of_dst == c
        srows = table_row[m]
        dloc = dst[m] - c * NPC
        bg = dloc >> 7
        s = bg // SPAN
        w = srows // WROWS
        key = s * NWIN + w
        order = np.lexsort((bg, key))
        srows, key, dloc, bg = srows[order], key[order], dloc[order], bg[order]
        cnts[c] = np.bincount(key, minlength=ncall)
        bin_sb = np.bincount(key * SPAN + (bg % SPAN),
                             minlength=ncall * SPAN).reshape(ncall, SPAN)
        cumh[c, :, 1:] = np.cumsum(bin_sb, axis=1)
        per_core.append((srows, key, dloc))

    # tiles per call: max over cores -> identical program on every core
    T_call = _ceil(cnts.max(axis=0), P)  # [ncall]
    gt0 = np.zeros(ncall + 1, dtype=np.int64)
    gt0[1:] = np.cumsum(T_call)
    TT = int(gt0[-1])

    # conservative per-(block, window) tile ranges shared by all cores
    # ranges[b][w] = (t0, t1) inclusive, or None
    ranges = [[None] * NWIN for _ in range(NBLK)]
    for s in range(NSPAN):
        b0 = s * SPAN
        for w in range(NWIN):
            call = s * NWIN + w
            for k in range(min(SPAN, NBLK - b0)):
                lo = int(cumh[:, call, k].min())
                hi = int(cumh[:, call, k + 1].max())
                if hi > lo:
                    ranges[b0 + k][w] = (int(gt0[call]) + lo // P,
                                         int(gt0[call]) + _ceil(hi, P) - 1)

    in_maps = []
    for c in range(N_CORES):
        srows, key, dloc = per_core[c]
        start = np.concatenate([[0], np.cumsum(cnts[c])[:-1]])
        pos = np.arange(len(key)) - start[key]
        gtile = gt0[key] + (pos >> 7)
        gpart = pos & 127

        # pad slots gather row 0 (harmless) and carry dst slot -1 (masked by
        # the is_equal selection matrix); avoids the negative-index strip path
        V = np.zeros((TT, P), np.int64)          # window-local source row
        D = np.full((TT, P), -1.0, np.float32)   # span-local dst slot
        V[gtile, gpart] = srows % WROWS
        D[gtile, gpart] = dloc - (key // NWIN) * (SPAN * P)

        # idx16 layout: per call the columns [8*gt0, 8*gt1); msg j (t-major)
        # lives at [16g + (j%16), gt0*8 + j//16], replicated to 128 partitions
        idx16 = np.zeros((P, TT * 8), np.int16)
        for call in range(ncall):
            a, b = int(gt0[call]), int(gt0[call + 1])
            if b == a:
                continue
            v = V[a:b, :].reshape(-1)
            blockv = v.reshape(-1, 16).T.astype(np.int16)
            idx16[:, a * 8:b * 8] = np.tile(blockv, (8, 1))

        xs = x[c * NPC:(c + 1) * NPC] * dinv[c * NPC:(c + 1) * NPC, None]
        xT = np.zeros((P, NPCP), np.float32)
        xT[:, :NPC] = xs.T
        dv = np.zeros(NPCP, np.float32)
        dv[:NPC] = dinv[c * NPC:(c + 1) * NPC]
        dinvT = np.ascontiguousarray(dv.reshape(NBLK, P).T)

        iot7 = np.broadcast_to(np.arange(SPAN * P, dtype=np.float32),
                               (P, SPAN * P)).copy()

        in_maps.append(
            {
                "xT": xT.astype(BF16),
                "idx16": idx16,
                "dsel": np.ascontiguousarray(D.T).astype(FP16),
                "dinvT": dinvT,
                "dinv2T": dinvT * dinvT,
                "w1": np.asarray(W1, np.float32).astype(BF16),
                "w2": np.asarray(W2, np.float32).astype(BF16),
                "iot7": iot7.astype(FP16),
                "ident": np.eye(P, dtype=np.float32).astype(BF16),
            }
        )
    return in_maps, T_call, ranges


# ---------------------------------------------------------------------------
# Device program
# ---------------------------------------------------------------------------

def build_program(T_call, ranges):
    n_f = FEAT
    gt0 = np.zeros(len(T_call) + 1, dtype=np.int64)
    gt0[1:] = np.cumsum(T_call)
    TT = int(gt0[-1])

    nc = bacc.Bacc("TRN2", target_bir_lowering=False, debug=False,
                   num_devices=N_CORES, num_swdge_queues=4)

    xT_d = nc.dram_tensor("xT", [P, NPCP], BF, kind="ExternalInput")
    idx16_d = nc.dram_tensor("idx16", [P, TT * 8], I16, kind="ExternalInput")
    dsel_d = nc.dram_tensor("dsel", [P, TT], F16, kind="ExternalInput")
    dinvT_d = nc.dram_tensor("dinvT", [P, NBLK], F32, kind="ExternalInput")
    dinv2T_d = nc.dram_tensor("dinv2T", [P, NBLK], F32, kind="ExternalInput")
    w_d = [nc.dram_tensor("w1", [n_f, n_f], BF, kind="ExternalInput"),
           nc.dram_tensor("w2", [n_f, n_f], BF, kind="ExternalInput")]
    iot7_d = nc.dram_tensor("iot7", [P, SPAN * P], F16, kind="ExternalInput")
    ident_d = nc.dram_tensor("ident", [P, P], BF, kind="ExternalInput")
    out_d = nc.dram_tensor("out", [NPC, 2 * n_f], F32, kind="ExternalOutput")

    s2_sh = nc.dram_tensor("s2sh", [NPCP, n_f], BF)
    hs_sh = [nc.dram_tensor(f"hs{L}sh", [NPCP, n_f], BF) for L in (1, 2)]
    hs_full = [nc.dram_tensor(f"hs{L}full", [NN, n_f], BF,
                              addr_space="Shared") for L in (1, 2)]
    groups = [list(range(N_CORES))]

    with tile.TileContext(nc) as tc:
        with (
            tc.tile_pool(name="const", bufs=1) as cpool,
            tc.tile_pool(name="big", bufs=1) as bigpool,
            tc.tile_pool(name="msg", bufs=3) as msgpool,
            tc.tile_pool(name="sel", bufs=10) as selpool,
            tc.tile_pool(name="post", bufs=6) as postpool,
            tc.tile_pool(name="psxw", bufs=2, space="PSUM") as psxw,
            tc.tile_pool(name="psag", bufs=6, space="PSUM") as psag,
        ):
            nc.gpsimd.load_library(mlp)
            w_t = []
            for L in (0, 1):
                wt = cpool.tile([n_f, n_f], BF, tag=f"w{L}", name=f"w{L}t")
                nc.sync.dma_start(out=wt[:], in_=w_d[L][:])
                w_t.append(wt)
            iot7_t = cpool.tile([P, SPAN * P], F16, tag="iot7", name="iot7_t")
            nc.sync.dma_start(out=iot7_t[:], in_=iot7_d[:])
            ident_t = cpool.tile([P, P], BF, tag="ident", name="ident_t")
            nc.sync.dma_start(out=ident_t[:], in_=ident_d[:])
            dinvT_t = cpool.tile([P, NBLK], F32, tag="dinvT", name="dinvT_t")
            nc.sync.dma_start(out=dinvT_t[:], in_=dinvT_d[:])
            dinv2T_t = cpool.tile([P, NBLK], F32, tag="dinv2T", name="dinv2T_t")
            nc.sync.dma_start(out=dinv2T_t[:], in_=dinv2T_d[:])

            # resident graph indices (shared by both layers)
            idx16_t = bigpool.tile([P, TT * 8], I16, tag="idx16", name="idx16_t")
            nc.sync.dma_start(out=idx16_t[:], in_=idx16_d[:])
            dsel_t = bigpool.tile([P, TT], F16, tag="dsel", name="dsel_t")
            nc.sync.dma_start(out=dsel_t[:], in_=dsel_d[:])

            # xT: layer-1 input, overwritten per span with transposed s2
            xT_t = bigpool.tile([P, NPCP], BF, tag="xT", name="xT_t")
            nc.sync.dma_start(out=xT_t[:], in_=xT_d[:])
            # resident own-shard hs (self-loop operand), overwritten per layer
            hso_t = bigpool.tile([P, NPCP], BF, tag="hso", name="hso_t")

            def xw_block(L, t):
                """hs_L[block t] = (xT[:, t].T @ W_L); store shard + SBUF copy.

                Layer-1 copies run on the (then idle) vector engine to shorten
                the startup ramp; layer-2 copies go to the scalar engine so
                they do not compete with the IS_EQ stream."""
                ps = psxw.tile([P, n_f], F32, tag="psxw", name="psxw_t")
                nc.tensor.matmul(out=ps[:], lhsT=xT_t[:, t * P:(t + 1) * P],
                                 rhs=w_t[L][:], start=True, stop=True)
                dst = hso_t[:, t * P:(t + 1) * P]
                if L == 0:
                    nc.vector.tensor_copy(out=dst, in_=ps[:])
                else:
                    nc.scalar.activation(out=dst, in_=ps[:],
                                         func=mybir.ActivationFunctionType.Copy)
                return nc.sync.dma_start(out=hs_sh[L][t * P:(t + 1) * P, :],
                                         in_=dst)

            def allgather_chunk(L, k, stores):
                ag = nc.gpsimd.collective_compute(
                    "AllGather", mybir.AluOpType.bypass, replica_groups=groups,
                    ins=[hs_sh[L][k * HALF:(k + 1) * HALF, :]],
                    outs=[hs_full[L][k * (NN // 2):(k + 1) * (NN // 2), :]])
                for s in stores:
                    add_dep_helper(ag.ins, s.ins, reason="allgather after hs stores")
                return ag

            glog = []  # gather instructions in emission order (for pinning)

            def span_gathers(L, s, ags, wins=range(NWIN), msg=None):
                """Issue window gather calls of span s (queue = window)."""
                t0 = int(gt0[s * NWIN])
                t1 = int(gt0[(s + 1) * NWIN])
                ts = t1 - t0
                if msg is None:
                    msg = msgpool.tile([P, ts, n_f], BF, tag="msg", name="msg_t")
                for w in wins:
                    a = int(gt0[s * NWIN + w])
                    b = int(gt0[s * NWIN + w + 1])
                    if b == a:
                        continue
                    nidx = (b - a) * P
                    g = nc.gpsimd.dma_gather(
                        msg[:, a - t0:b - t0, :],
                        hs_full[L][(w * WROWS):(w * WROWS + WROWS), :],
                        idx16_t[:, a * 8:b * 8],
                        nidx, nidx, n_f, single_packet=False, queue_num=w)
                    add_dep_helper(g.ins, ags[w // 2].ins,
                                   reason="gather after allgather chunk")
                    glog.append(g)
                return msg, t0

            def span_agg(L, s, msg, t0, s2_stores):
                """Segment-sum + postprocess the 7 blocks of span s."""
                b0 = s * SPAN
                for k in range(min(SPAN, NBLK - b0)):
                    b = b0 + k
                    rlist = [ranges[b][w] for w in range(NWIN)
                             if ranges[b][w] is not None]
                    ps = psag.tile([P, n_f], F32, tag="psag", name="psag_t")
                    nmm = sum(r1 - r0 + 1 for r0, r1 in rlist)
                    nc.tensor.matmul(out=ps[:], lhsT=ident_t[:],
                                     rhs=hso_t[:, b * P:(b + 1) * P],
                                     start=True, stop=(nmm == 0))
                    j = 0
                    for r0, r1 in rlist:
                        rn = r1 - r0 + 1
                        sel = selpool.tile([P, rn, P], BF, tag="sel",
                                           name="sel_t")
                        nc.vector.tensor_tensor(
                            out=sel[:],
                            in0=iot7_t[:, None, k * P:(k + 1) * P]
                                .to_broadcast([P, rn, P]),
                            in1=dsel_t[:, r0:r1 + 1, None]
                                .to_broadcast([P, rn, P]),
                            op=mybir.AluOpType.is_equal)
                        for t in range(rn):
                            nc.tensor.matmul(out=ps[:],
                                             lhsT=sel[:, t, :],
                                             rhs=msg[:, r0 + t - t0, :],
                                             start=False,
                                             stop=(j == nmm - 1))
                            j += 1
                    # h = relu(dinv * agg); s2 = dinv * h = relu(dinv^2 * agg)
                    h_t = postpool.tile([P, n_f], F32, tag="hrelu",
                                        name="hrelu_t")
                    nc.scalar.activation(out=h_t[:], in_=ps[:],
                                         func=mybir.ActivationFunctionType.Relu,
                                         scale=dinvT_t[:, b:b + 1])
                    rows = min(P, NPC - b * P)
                    nc.scalar.dma_start(
                        out=out_d[b * P:b * P + rows, L * n_f:(L + 1) * n_f],
                        in_=h_t[:rows, :])
                    if L == 0:
                        s2_t = postpool.tile([P, n_f], BF, tag="s2",
                                             name="s2_t")
                        nc.scalar.activation(
                            out=s2_t[:], in_=ps[:],
                            func=mybir.ActivationFunctionType.Relu,
                            scale=dinv2T_t[:, b:b + 1])
                        s2_stores.append(
                            nc.sync.dma_start(out=s2_sh[b * P:(b + 1) * P, :],
                                              in_=s2_t[:]))

            def span_xw2(s, s2_stores, hs2_stores):
                """Transpose span s's s2 back into xT and run its hs2 matmuls."""
                b0, b1 = s * SPAN, min((s + 1) * SPAN, NBLK)
                tr = nc.sync.dma_start_transpose(
                    out=xT_t[:, b0 * P:b1 * P],
                    in_=s2_sh[b0 * P:b1 * P, :])
                for st in s2_stores:
                    add_dep_helper(tr.ins, st.ins, reason="transpose after s2")
                for t in range(b0, b1):
                    hs2_stores.append(xw_block(1, t))

            # ---- layer 1 dense matmuls + chunked AllGather ----
            st1 = [xw_block(0, t) for t in range(NBLK)]
            ag1 = [allgather_chunk(0, 0, st1[:NBLK // 2]),
                   allgather_chunk(0, 1, st1[NBLK // 2:])]

            # ---- layer 1 aggregation, with layer-2 xw pipelined per span ----
            ag2 = [None, None]
            hs2_stores = []
            pending = []  # (s2_stores of span) awaiting span_xw2
            for s in range(NSPAN):
                msg, t0 = span_gathers(0, s, ag1)
                s2st = []
                span_agg(0, s, msg, t0, s2st)
                pending.append((s, s2st))
                # run xw2 for the previous span (keeps PE from stalling on
                # the s2 DRAM round-trip)
                if len(pending) > 1:
                    ps, pst = pending.pop(0)
                    span_xw2(ps, pst, hs2_stores)
            for ps, pst in pending:
                span_xw2(ps, pst, hs2_stores)

            # ---- layer 2 aggregation ----
            # Both AllGather chunks are emitted after the last layer-1
            # gathers (chunk 0's inputs are long since stored, so it only
            # costs its own execution, overlapped with layer-1 tail work).
            # The first two spans issue their window-0/1 gathers before any
            # window-2/3 gather so chunk 1 completes behind real gather work.
            # Both chunks pinned behind the last layer-1 gather (the scheduler
            # would otherwise hoist them mid-layer-1 and stall gather
            # dispatch); triggered back-to-back so their rendezvous/transfer
            # overlap as much as the CC hardware allows.
            last_l1 = glog[-1]
            ag2[0] = allgather_chunk(1, 0, hs2_stores[:NBLK // 2])
            add_dep_helper(ag2[0].ins, last_l1.ins,
                           reason="pin ag2[0] after last layer-1 gather")
            ag2[1] = allgather_chunk(1, 1, hs2_stores[NBLK // 2:])
            add_dep_helper(ag2[1].ins, last_l1.ins,
                           reason="pin ag2[1] after last layer-1 gather")
            m0, t00 = span_gathers(1, 0, ag2, wins=(0, 1))
            m1, t01 = span_gathers(1, 1, ag2, wins=(0, 1))
            span_gathers(1, 0, ag2, wins=(2, 3), msg=m0)
            span_gathers(1, 1, ag2, wins=(2, 3), msg=m1)
            span_agg(1, 0, m0, t00, [])
            span_agg(1, 1, m1, t01, [])
            for s in range(2, NSPAN):
                msg, t0 = span_gathers(1, s, ag2)
                span_agg(1, s, msg, t0, [])

    nc.compile()
    return nc


# ---------------------------------------------------------------------------
# Entry point
# ---------------------------------------------------------------------------

_CACHE: dict = {}


def _install_ntff_hook():
    """Wire the axon NTFF profiling hook that this image leaves unplugged.

    Harness-side instrumentation only; no-op when already present or
    when the pieces are missing."""
    try:
        from antenv.axon_hooks import get_axon_ntff_profile_hook  # noqa: F401
        return
    except ImportError:
        pass
    try:
        import sys
        import types

        if "/root/.axon_site" not in sys.path:
            sys.path.insert(0, "/root/.axon_site")
        from trn_agent_boot.trn_boot import _ntff_profile_via_ctypes

        hook = _ntff_profile_via_ctypes("/opt/axon/libaxon_pjrt.so")
        import antenv

        m = types.ModuleType("antenv.axon_hooks")
        m.get_axon_ntff_profile_hook = lambda: hook
        m.set_axon_ntff_profile_hook = lambda h: None
        sys.modules["antenv.axon_hooks"] = m
        antenv.axon_hooks = m
        import concourse.bass_utils as bu

        bu.upload_artifacts = lambda tmpdir: f"local:{tmpdir}"
    except Exception as e:  # degrade to no tracing
        print("ntff hook install failed:", e)


def run(cfg, inputs: dict, trace: bool = False):
    if trace:
        _install_ntff_hook()
    in_maps, T_call, ranges = prep_inputs(**inputs)
    key = (T_call.tobytes(), str(ranges))
    if key not in _CACHE:
        _CACHE[key] = build_program(T_call, ranges)
    nc = _CACHE[key]
    res = run_bass_kernel_spmd(nc, in_maps, list(range(N_CORES)), trace=trace)
    out = np.concatenate([res.results[c]["out"] for c in range(N_CORES)], axis=0)
    return out, res


def kernel(**inputs) -> np.ndarray:
    trace = bool(os.environ.get("BASS_TRACE"))
    try:
        out, _ = run(CFG, inputs, trace=trace)
    except Exception:
        # transient NRT / device hiccups happen rarely; one retry
        out, _ = run(CFG, inputs, trace=trace)
    return out
